# revision 15
# baseline (speedup 1.0000x reference)
"""Trainium2 Bass kernel for the binarized BasicBlock (dense_cnn).

Contract: kernel(**inputs) takes the FULL unsharded inputs (numpy arrays,
keyed as in reference.setup_inputs()) and returns the FULL output
(32, 128, 56, 56) float32.  Internally shards the batch dim across 8
NeuronCores (pure data parallel, params replicated).

Per-core layout: 4 images as 2 pairs; each pair in 2 half-height units of
28 output rows.  Partitions hold (imgA ch0-63 | imgB ch0-63) for stage-1
tensors.  Design notes (v2):
 - conv1 = 9 shifted matmuls per psum chunk, A/B images on concurrent
   64x64 PE quadrants writing one shared psum tile; the 4*s3 binary-weight
   scale is folded into the bf16 tap weights (exact: bf16(4*s3)*int sums
   stay exact in fp32 psum).
 - residual avgpool on DVE in fp32 (exact, so sign2 never flips); one
   tensor_tensor per chunk merges psum+pool into u.
 - sign ops use a u16 bit-trick on DVE (2x mode) where biases are zero;
   ACT handles a tunable share of sign1 plus the PReLUs, batched to one
   big-N instruction per unit to amortize the ~293ns/inst ACT overhead.
 - stage2: per chunk one [128,1024] psum pair-tile (A bank0 / B bank1),
   matmuls interleaved A/B for quadrant concurrency, one strided ACT
   Prelu over both images.
 - stage2 chunks of unit k-1 are emitted interleaved into unit k's conv
   stream so the PE never idles (stays HAM-warm at 2.4GHz); x slabs
   rotate through 3 buffers so input DMA runs 2-3 units ahead.
"""
import sys

sys.path.insert(0, "/opt/trn_rl_repo")

import numpy as np
import ml_dtypes

import concourse.bacc as bacc
import concourse.mybir as mybir
import concourse.tile as tile
from concourse import bass_utils

# Problem shapes (hardcoded per spec)
B, CIN, H, W = 32, 64, 112, 112
COUT = 2 * CIN
NCORES = 8
BPC = B // NCORES          # images per core = 4
NPAIR = BPC // 2           # image pairs per core = 2
OH, OW = H // 2, W // 2    # 56, 56
HALF = OH // 2             # 28 output rows per unit
NCHUNK = 4                 # psum chunks per unit (7 out rows each)
CROWS = HALF // NCHUNK     # 7
CN = CROWS * OW            # 392 cols per chunk
UN = HALF * OW             # 1568 elems per unit (per partition)
SROWS = 57                 # raw/sign slab rows (input rows 2*oy0-1 .. 2*oy0+55)
SPITCH = 114               # sign slab col pitch (1 left pad + 112 + 1 right pad)
NA_ACT = 34                # sign1 rows handled by ACT (rest on DVE bit-trick)

# param columns
PA1, PB12, PB11, PA2F, PB22F, PS2V, PBS2, PB13, PB23F = range(9)
NPARAM = 9
# weight blocks of 64 cols: conv taps 0..8 (ky*3+kx) pre-scaled by 4*s3;
# then two 128-wide blocks: [wpw1|wpw2] and [diag1|diag2] for M=128
# stage-2 matmuls
NBLK = 9
WCOLS = NBLK * 64 + 256
O_PW = NBLK * 64          # [wpw1|wpw2] at cols O_PW:O_PW+128
O_DIAG = NBLK * 64 + 128  # [diag1|diag2]

_cache = {}


def _build(scal, reps=1):
    """Build the bass program. scal: host-derived scalars/flags.
    reps>1 replicates the whole compute (legacy knob, kept for test.py)."""
    nc = bacc.Bacc("TRN2", target_bir_lowering=False, debug=False)
    f32 = mybir.dt.float32
    bf16 = mybir.dt.bfloat16
    u16 = mybir.dt.uint16
    AF = mybir.ActivationFunctionType
    ALU = mybir.AluOpType

    s3x4 = scal["s3x4"]
    b11_zero = scal["b11_zero"]
    fast_sign2 = scal["fast_sign2"]
    trick_sign2 = scal["trick_sign2"]
    has_b13 = scal["has_b13"]
    has_b23 = scal["has_b23"]

    tc_cm = tile.TileContext(nc)
    tc = tc_cm.__enter__()
    dram_cm = tc.tile_pool(name="dram", bufs=1, space="DRAM")
    dram = dram_cm.__enter__()

    x_d = dram.tile([BPC, CIN, H, W], f32, kind="ExternalInput")
    w_d = dram.tile([128, WCOLS], bf16, kind="ExternalInput")
    p_d = dram.tile([128, NPARAM], f32, kind="ExternalInput")
    y_d = dram.tile([BPC, COUT, OH, OW], f32, kind="ExternalOutput")

    pools = []

    def pool(name, **kw):
        cm = tc.tile_pool(name=name, **kw)
        pools.append(cm)
        return cm.__enter__()

    const = pool("const", bufs=1)
    pers = pool("pers", bufs=1)
    slab = pool("slab", bufs=3)
    work = pool("work", bufs=2)
    psum = pool("psum", bufs=2, space="PSUM")
    psum2 = pool("psum2", bufs=2, space="PSUM")

    wt = const.tile([128, WCOLS], bf16)
    pt = const.tile([128, NPARAM], f32)
    nc.sync.dma_start(wt[:], w_d[:])
    nc.sync.dma_start(pt[:], p_d[:])

    # persistent sign slabs: index by half h (stable pad semantics)
    sp = [pers.tile([128, SROWS * SPITCH], bf16, tag=f"sp{h}", name=f"sp{h}")
          for h in range(2)]
    for h in range(2):
        spv0 = sp[h][:].rearrange("p (r c) -> p r c", r=SROWS)
        nc.vector.memset(spv0[:, 0:1, :], 0.0)
        nc.vector.memset(spv0[:, :, 0:1], 0.0)
        nc.vector.memset(spv0[:, :, 113:114], 0.0)

    def wap(blk):
        return wt[:, 64 * blk:64 * blk + 64]

    units = [(p, h) for _ in range(reps)
             for p in range(NPAIR) for h in range(2)]
    xps = {}
    s4s = {}

    def emit_load(k):
        if k >= len(units) or k in xps:
            return
        p, h = units[k]
        nA = 2 * p
        oy0 = HALF * h
        r0 = 2 * oy0 - 1           # input row of slab row 0
        ld0 = 1 if h == 0 else 0   # first valid slab row
        xp = slab.tile([128, SROWS * W], f32, tag="xp", name=f"xp{k}")
        xpv = xp[:].rearrange("p (r c) -> p r c", r=SROWS)
        # k==0: band-split load so the first sign1/conv starts early
        bands = ([(ld0, 15), (15, 29), (29, 43), (43, SROWS)] if k == 0
                 else [(ld0, SROWS)])
        for (ra, rb) in bands:
            src = x_d[nA:nA + 2, :, r0 + ra:r0 + rb, :].rearrange(
                "i c r w -> (i c) r w")
            nc.sync.dma_start(xpv[:, ra:rb, :], src)
        xps[k] = (xp, bands)

    def emit_pre(k):
        """sign1 -> sp and avgpool -> s4 for unit k."""
        if k >= len(units) or k in s4s:
            return
        p, h = units[k]
        ld0 = 1 if h == 0 else 0
        xp, bands = xps[k]
        xpv = xp[:].rearrange("p (r c) -> p r c", r=SROWS)
        spv = sp[h][:].rearrange("p (r c) -> p r c", r=SROWS)

        if k == 0:
            # per-band ACT sign for lowest first-conv latency
            for (ra, rb) in bands:
                nc.scalar.activation(
                    spv[:, ra:rb, 1:113], xpv[:, ra:rb, :],
                    AF.Sign, bias=pt[:, PB11:PB11 + 1])
        elif not b11_zero:
            nc.scalar.activation(
                spv[:, ld0:SROWS, 1:113], xpv[:, ld0:SROWS, :],
                AF.Sign, bias=pt[:, PB11:PB11 + 1])
        else:
            # split ACT / DVE(u16 bit-trick, 2x mode) to balance engines
            na = ld0 + NA_ACT
            nc.scalar.activation(
                spv[:, ld0:na, 1:113], xpv[:, ld0:na, :], AF.Sign)
            nr = SROWS - na
            xhi = xp[:, na * W:SROWS * W].bitcast(u16).rearrange(
                "p (r c two) -> p r c two", r=nr, two=2)[:, :, :, 1:2]
            sout = spv[:, na:SROWS, 1:113].bitcast(u16)
            nc.vector.tensor_scalar(
                sout, xhi, 0x8000, 0x3F80,
                ALU.bitwise_and, ALU.bitwise_or)

        # avgpool x4 (fp32 exact): row-sum on GpSimd in 4 slices (bounds the
        # DVE/GpSimd shared-port lock holds), col-sum on DVE in 2 halves
        prow = work.tile([128, HALF * W], f32, tag="prow", name="prow")
        prv = prow[:].rearrange("p (r c) -> p r c", r=HALF)
        s4 = work.tile([128, UN], f32, tag="s4", name="s4")
        s4v = s4[:].rearrange("p (r c) -> p r c", r=HALF)
        for q in range(4):
            # GpSimd is ~2.6x slower per element than DVE here; give it one
            # slice (it is otherwise idle), DVE the rest
            eng = nc.gpsimd if q == 0 else nc.vector
            ra, rb = 7 * q, 7 * q + 7
            eng.tensor_tensor(
                prv[:, ra:rb, :],
                xpv[:, 1 + 2 * ra:1 + 2 * rb:2, :],
                xpv[:, 2 + 2 * ra:2 * rb + 1:2, :], ALU.add)
        for q in range(2):
            ra, rb = 14 * q, 14 * q + 14
            nc.vector.tensor_tensor(
                s4v[:, ra:rb, :], prv[:, ra:rb, 0:W:2],
                prv[:, ra:rb, 1:W:2], ALU.add)
        s4s[k] = s4

    for k in range(3):
        emit_load(k)
    emit_pre(0)

    pending = []  # deferred stage2/prelu2/store emitters from previous unit

    for k, (p, h) in enumerate(units):
        nA, nB = 2 * p, 2 * p + 1
        oy0 = HALF * h
        s4 = s4s.pop(k)
        spv = sp[h][:].rearrange("p (r c) -> p r c", r=SROWS)

        # ---- conv1: 9 taps x 4 chunks, A/B on concurrent 64x64 quads;
        # stage2 chunks of unit k-1 interleave to keep PE dense ----
        u = work.tile([128, UN], f32, tag="u", name="u")
        out1 = work.tile([128, UN], bf16, tag="out1", name="out1")
        sg2 = work.tile([128, UN], bf16, tag="sg2", name="sg2")
        for half in range(2):
            # chunk-pair psum: chunk 2h at bank 0, chunk 2h+1 at bank 1
            cp = psum.tile([128, 1024], f32, tag="ps", name="ps")
            for cc in range(2):
                c = 2 * half + cc
                for t in range(9):
                    ky, kx = divmod(t, 3)
                    rs = ky + 14 * c
                    for i in range(2):
                        pr = slice(64 * i, 64 * i + 64)
                        rhs = spv[pr, rs:rs + 13:2, kx:kx + 111:2]
                        nc.tensor.matmul(
                            cp[pr, 512 * cc:512 * cc + CN],
                            wap(t)[pr, :], rhs,
                            start=(t == 0), stop=(t == 8),
                        )
                if c == 0:
                    emit_pre(k + 1)
                if pending:
                    pending.pop(0)()
            hs = slice(2 * CN * half, 2 * CN * (half + 1))
            cpv = cp[:].rearrange("p (i n) -> p i n", i=2)[:, :, 0:CN]
            uv = u[:, hs].rearrange("p (i n) -> p i n", i=2)
            s4h = s4[:, hs].rearrange("p (i n) -> p i n", i=2)
            nc.vector.scalar_tensor_tensor(
                uv, cpv, s3x4, s4h, ALU.mult, ALU.add)
            # prelu1/sign2 per half-unit: shorter chains into stage2
            nc.scalar.activation(
                out1[:, hs], u[:, hs], AF.Prelu,
                bias=pt[:, PB12:PB12 + 1], scale=0.25,
                alpha=pt[:, PA1:PA1 + 1])
            if trick_sign2:
                uhi = u[:, hs].bitcast(u16).rearrange(
                    "p (n two) -> p n two", two=2)[:, :, 1:2]
                nc.vector.tensor_scalar(
                    sg2[:, hs].bitcast(u16), uhi, 0x8000, 0x3F80,
                    ALU.bitwise_and, ALU.bitwise_or)
            elif fast_sign2:
                nc.scalar.activation(
                    sg2[:, hs], u[:, hs], AF.Sign,
                    bias=pt[:, PB12:PB12 + 1], scale=0.25)
        while pending:
            pending.pop(0)()

        if has_b13:
            nc.vector.tensor_scalar(
                out1[:], out1[:], pt[:, PB13:PB13 + 1], None, ALU.add)
        if not fast_sign2 and not trick_sign2:
            nc.scalar.activation(
                sg2[:], out1[:], AF.Sign, bias=pt[:, PBS2:PBS2 + 1])

        emit_load(k + 3)

        # ---- stage 2 (deferred): per chunk a [128,1024] psum pair-tile
        # (A @ cols 0:CN, B @ cols 512:512+CN), matmuls interleaved A/B ----
        stg = work.tile([128, 2 * UN], f32, tag="stg", name="stg")

        def mk_stage2(c, k=k, out1=out1, sg2=sg2, stg=stg,
                      nA=nA, nB=nB, oy0=oy0):
            def emit():
                cs = slice(CN * c, CN * (c + 1))
                p2 = psum2.tile([128, 1024], f32, tag="ps2", name="ps2")
                for i in range(2):
                    pr = slice(64 * i, 64 * i + 64)
                    o2 = slice(512 * i, 512 * i + CN)
                    nc.tensor.matmul(
                        p2[:, o2], wt[pr, O_PW:O_PW + 128], sg2[pr, cs],
                        start=True, stop=False)
                for i in range(2):
                    pr = slice(64 * i, 64 * i + 64)
                    o2 = slice(512 * i, 512 * i + CN)
                    nc.tensor.matmul(
                        p2[:, o2], wt[pr, O_DIAG:O_DIAG + 128], out1[pr, cs],
                        start=False, stop=True)
                # one strided ACT Prelu over both images
                pin = p2[:].rearrange("p (i n) -> p i n", i=2)[:, :, 0:CN]
                pout = stg[:].rearrange("p (i n) -> p i n", i=2)[:, :, cs]
                nc.scalar.activation(
                    pout, pin, AF.Prelu,
                    bias=pt[:, PB22F:PB22F + 1],
                    scale=pt[:, PS2V:PS2V + 1],
                    alpha=pt[:, PA2F:PA2F + 1])
                if has_b23 and c == NCHUNK - 1:
                    nc.vector.tensor_scalar(
                        stg[:], stg[:], pt[:, PB23F:PB23F + 1],
                        None, ALU.add)
                # store per half-unit per image (overlap, good desc size);
                # with b23 the add covers all rows, so store only at the end
                if has_b23:
                    rr = (0, HALF) if c == NCHUNK - 1 else None
                else:
                    rr = {1: (0, 14), NCHUNK - 1: (14, HALF)}.get(c)
                if rr is not None:
                    for i, n in enumerate((nA, nB)):
                        sv = stg[:, UN * i:UN * (i + 1)].rearrange(
                            "p (r c) -> p r c", r=HALF)
                        nc.sync.dma_start(
                            y_d[n, :, oy0 + rr[0]:oy0 + rr[1], :],
                            sv[:, rr[0]:rr[1], :])
            return emit

        pending = [mk_stage2(c) for c in range(NCHUNK)]

    while pending:
        pending.pop(0)()

    for cm in reversed(pools):
        cm.__exit__(None, None, None)
    dram_cm.__exit__(None, None, None)
    tc_cm.__exit__(None, None, None)
    nc.compile()
    return nc, x_d.name, w_d.name, p_d.name, y_d.name


def _prep(inputs):
    f32 = np.float32
    bf = ml_dtypes.bfloat16
    w3 = np.asarray(inputs["w3"], f32)
    wpw1 = np.asarray(inputs["wpw1"], f32)
    wpw2 = np.asarray(inputs["wpw2"], f32)
    a1 = np.asarray(inputs["a1"], f32).reshape(CIN)
    a2 = np.asarray(inputs["a2"], f32).reshape(COUT)
    b11 = np.asarray(inputs["b11"], f32).reshape(CIN)
    b12 = np.asarray(inputs["b12"], f32).reshape(CIN)
    b13 = np.asarray(inputs["b13"], f32).reshape(CIN)
    b21 = np.asarray(inputs["b21"], f32).reshape(CIN)
    b22 = np.asarray(inputs["b22"], f32).reshape(COUT)
    b23 = np.asarray(inputs["b23"], f32).reshape(COUT)

    s3 = float(np.mean(np.abs(w3))) or 1.0
    s1 = float(np.mean(np.abs(wpw1))) or 1.0
    s2 = float(np.mean(np.abs(wpw2))) or 1.0

    # diag entries bf16(1/s_j); prelu2 scale 1/d_j compensates the rounding
    d1 = float(bf(1.0 / s1))
    d2 = float(bf(1.0 / s2))

    whalf = np.zeros((64, WCOLS), f32)
    sgn = np.sign
    for t in range(9):
        ky, kx = divmod(t, 3)
        whalf[:, 64 * t:64 * t + 64] = sgn(w3[:, :, ky, kx]).T
    whalf[:, O_PW:O_PW + 64] = sgn(wpw1[:, :, 0, 0]).T
    whalf[:, O_PW + 64:O_PW + 128] = sgn(wpw2[:, :, 0, 0]).T
    whalf[:, O_DIAG:O_DIAG + 64] = d1 * np.eye(64, dtype=f32)
    whalf[:, O_DIAG + 64:O_DIAG + 128] = d2 * np.eye(64, dtype=f32)
    wfull = np.concatenate([whalf, whalf], axis=0).astype(bf)

    def pairc(v):  # channel vec (64,) -> pair-layout (128,)
        return np.concatenate([v, v])

    params = np.zeros((128, NPARAM), f32)
    params[:, PA1] = pairc(a1)
    params[:, PB12] = pairc(b12)
    params[:, PB11] = pairc(b11)
    params[:, PA2F] = a2
    params[:, PB22F] = b22
    params[:, PS2V] = np.concatenate(
        [np.full(64, 1.0 / d1, f32), np.full(64, 1.0 / d2, f32)])
    params[:, PBS2] = pairc(b13 + b21)
    params[:, PB13] = pairc(b13)
    params[:, PB23F] = b23

    fast_sign2 = bool(np.all(b13 + b21 == 0.0) and np.all(a1 > 0))
    scal = {
        "s3x4": 4.0 * s3,
        "fast_sign2": fast_sign2,
        "trick_sign2": bool(fast_sign2 and np.all(b12 == 0.0)),
        "b11_zero": bool(np.all(b11 == 0.0)),
        "has_b13": bool(np.any(b13 != 0.0)),
        "has_b23": bool(np.any(b23 != 0.0)),
    }
    return wfull, params, scal


def kernel(**inputs):
    x = np.ascontiguousarray(np.asarray(inputs["x"], np.float32))
    wfull, params, scal = _prep(inputs)

    key = tuple(sorted(scal.items())) + (float(params.sum()),)
    if key not in _cache:
        _cache.clear()
        _cache[key] = _build(scal)
    nc, xn, wn, pn, yn = _cache[key]

    in_maps = []
    for i in range(NCORES):
        in_maps.append({
            xn: np.ascontiguousarray(x[BPC * i:BPC * (i + 1)]),
            wn: wfull,
            pn: params,
        })
    res = bass_utils.run_bass_kernel_spmd(nc, in_maps, core_ids=list(range(NCORES)))
    out = np.concatenate([res.results[i][yn] for i in range(NCORES)], axis=0)
    return out.astype(np.float32)


# revision 20
# speedup vs baseline: 1.0304x; 1.0304x over previous
"""Trainium2 Bass kernel for the binarized BasicBlock (dense_cnn).

Contract: kernel(**inputs) takes the FULL unsharded inputs (numpy arrays,
keyed as in reference.setup_inputs()) and returns the FULL output
(32, 128, 56, 56) float32.  Internally shards the batch dim across 8
NeuronCores (pure data parallel, params replicated).

Per-core layout: 4 images as 2 pairs; each pair in 2 half-height units of
28 output rows.  Partitions hold (imgA ch0-63 | imgB ch0-63) for stage-1
tensors.  Design notes (v2):
 - conv1 = 9 shifted matmuls per psum chunk, A/B images on concurrent
   64x64 PE quadrants writing one shared psum tile; the 4*s3 binary-weight
   scale is folded into the bf16 tap weights (exact: bf16(4*s3)*int sums
   stay exact in fp32 psum).
 - residual avgpool on DVE in fp32 (exact, so sign2 never flips); one
   tensor_tensor per chunk merges psum+pool into u.
 - sign ops use a u16 bit-trick on DVE (2x mode) where biases are zero;
   ACT handles a tunable share of sign1 plus the PReLUs, batched to one
   big-N instruction per unit to amortize the ~293ns/inst ACT overhead.
 - stage2: per chunk one [128,1024] psum pair-tile (A bank0 / B bank1),
   matmuls interleaved A/B for quadrant concurrency, one strided ACT
   Prelu over both images.
 - stage2 chunks of unit k-1 are emitted interleaved into unit k's conv
   stream so the PE never idles (stays HAM-warm at 2.4GHz); x slabs
   rotate through 3 buffers so input DMA runs 2-3 units ahead.
"""
import sys

sys.path.insert(0, "/opt/trn_rl_repo")

import numpy as np
import ml_dtypes

import concourse.bacc as bacc
import concourse.mybir as mybir
import concourse.tile as tile
from concourse import bass_utils

# Problem shapes (hardcoded per spec)
B, CIN, H, W = 32, 64, 112, 112
COUT = 2 * CIN
NCORES = 8
BPC = B // NCORES          # images per core = 4
NPAIR = BPC // 2           # image pairs per core = 2
OH, OW = H // 2, W // 2    # 56, 56
HALF = OH // 2             # 28 output rows per unit
NCHUNK = 4                 # psum chunks per unit (7 out rows each)
CROWS = HALF // NCHUNK     # 7
CN = CROWS * OW            # 392 cols per chunk
UN = HALF * OW             # 1568 elems per unit (per partition)
SROWS = 57                 # raw/sign slab rows (input rows 2*oy0-1 .. 2*oy0+55)
SPITCH = 114               # sign slab col pitch (1 left pad + 112 + 1 right pad)
NA_ACT = 34                # sign1 rows handled by ACT (rest on DVE bit-trick)

# param columns
PA1, PB12, PB11, PA2F, PB22F, PS2V, PBS2, PB13, PB23F = range(9)
NPARAM = 9
# weight blocks of 64 cols: conv taps 0..8 (ky*3+kx) pre-scaled by 4*s3;
# then two 128-wide blocks: [wpw1|wpw2] and [diag1|diag2] for M=128
# stage-2 matmuls
NBLK = 9
WCOLS = NBLK * 64 + 256
O_PW = NBLK * 64          # [wpw1|wpw2] at cols O_PW:O_PW+128
O_DIAG = NBLK * 64 + 128  # [diag1|diag2]

_cache = {}


def _build(scal, reps=1):
    """Build the bass program. scal: host-derived scalars/flags.
    reps>1 replicates the whole compute (legacy knob, kept for test.py)."""
    nc = bacc.Bacc("TRN2", target_bir_lowering=False, debug=False)
    f32 = mybir.dt.float32
    bf16 = mybir.dt.bfloat16
    u16 = mybir.dt.uint16
    AF = mybir.ActivationFunctionType
    ALU = mybir.AluOpType

    s3x4 = scal["s3x4"]
    b11_zero = scal["b11_zero"]
    fast_sign2 = scal["fast_sign2"]
    trick_sign2 = scal["trick_sign2"]
    has_b13 = scal["has_b13"]
    has_b23 = scal["has_b23"]

    tc_cm = tile.TileContext(nc)
    tc = tc_cm.__enter__()
    dram_cm = tc.tile_pool(name="dram", bufs=1, space="DRAM")
    dram = dram_cm.__enter__()

    x_d = dram.tile([BPC, CIN, H, W], f32, kind="ExternalInput")
    w_d = dram.tile([128, WCOLS], bf16, kind="ExternalInput")
    p_d = dram.tile([128, NPARAM], f32, kind="ExternalInput")
    y_d = dram.tile([BPC, COUT, OH, OW], f32, kind="ExternalOutput")

    pools = []

    def pool(name, **kw):
        cm = tc.tile_pool(name=name, **kw)
        pools.append(cm)
        return cm.__enter__()

    const = pool("const", bufs=1)
    pers = pool("pers", bufs=1)
    slab = pool("slab", bufs=3)
    work = pool("work", bufs=2)
    work3 = pool("work3", bufs=3)
    psum = pool("psum", bufs=2, space="PSUM")
    psum2 = pool("psum2", bufs=2, space="PSUM")

    wt = const.tile([128, WCOLS], bf16)
    pt = const.tile([128, NPARAM], f32)
    nc.sync.dma_start(wt[:], w_d[:])
    nc.sync.dma_start(pt[:], p_d[:])

    # persistent sign slabs: index by half h (stable pad semantics)
    sp = [pers.tile([128, SROWS * SPITCH], bf16, tag=f"sp{h}", name=f"sp{h}")
          for h in range(2)]
    for h in range(2):
        spv0 = sp[h][:].rearrange("p (r c) -> p r c", r=SROWS)
        nc.vector.memset(spv0[:, 0:1, :], 0.0)
        nc.vector.memset(spv0[:, :, 0:1], 0.0)
        nc.vector.memset(spv0[:, :, 113:114], 0.0)

    def wap(blk):
        return wt[:, 64 * blk:64 * blk + 64]

    units = [(p, h) for _ in range(reps)
             for p in range(NPAIR) for h in range(2)]
    xps = {}
    s4s = {}

    def emit_load(k):
        if k >= len(units) or k in xps:
            return
        p, h = units[k]
        nA = 2 * p
        oy0 = HALF * h
        r0 = 2 * oy0 - 1           # input row of slab row 0
        ld0 = 1 if h == 0 else 0   # first valid slab row
        xp = slab.tile([128, SROWS * W], f32, tag="xp", name=f"xp{k}")
        xpv = xp[:].rearrange("p (r c) -> p r c", r=SROWS)
        # k==0: band-split load so the first sign1/conv starts early
        bands = ([(ld0, 15), (15, 29), (29, 43), (43, SROWS)] if k == 0
                 else [(ld0, SROWS)])
        for (ra, rb) in bands:
            src = x_d[nA:nA + 2, :, r0 + ra:r0 + rb, :].rearrange(
                "i c r w -> (i c) r w")
            nc.sync.dma_start(xpv[:, ra:rb, :], src)
        xps[k] = (xp, bands)

    def emit_pre(k):
        """sign1 -> sp and avgpool -> s4 for unit k."""
        if k >= len(units) or k in s4s:
            return
        p, h = units[k]
        ld0 = 1 if h == 0 else 0
        xp, bands = xps[k]
        xpv = xp[:].rearrange("p (r c) -> p r c", r=SROWS)
        spv = sp[h][:].rearrange("p (r c) -> p r c", r=SROWS)

        if k == 0:
            # per-band ACT sign for lowest first-conv latency
            for (ra, rb) in bands:
                nc.scalar.activation(
                    spv[:, ra:rb, 1:113], xpv[:, ra:rb, :],
                    AF.Sign, bias=pt[:, PB11:PB11 + 1])
        elif not b11_zero:
            nc.scalar.activation(
                spv[:, ld0:SROWS, 1:113], xpv[:, ld0:SROWS, :],
                AF.Sign, bias=pt[:, PB11:PB11 + 1])
        else:
            # split ACT / DVE(u16 bit-trick, 2x mode) to balance engines
            na = ld0 + NA_ACT
            nc.scalar.activation(
                spv[:, ld0:na, 1:113], xpv[:, ld0:na, :], AF.Sign)
            nr = SROWS - na
            xhi = xp[:, na * W:SROWS * W].bitcast(u16).rearrange(
                "p (r c two) -> p r c two", r=nr, two=2)[:, :, :, 1:2]
            sout = spv[:, na:SROWS, 1:113].bitcast(u16)
            nc.vector.tensor_scalar(
                sout, xhi, 0x8000, 0x3F80,
                ALU.bitwise_and, ALU.bitwise_or)

        # avgpool x4 (fp32 exact): row-sum on GpSimd in 4 slices (bounds the
        # DVE/GpSimd shared-port lock holds), col-sum on DVE in 2 halves
        prow = work.tile([128, HALF * W], f32, tag="prow", name="prow")
        prv = prow[:].rearrange("p (r c) -> p r c", r=HALF)
        s4 = work3.tile([128, UN], f32, tag="s4", name="s4")
        s4v = s4[:].rearrange("p (r c) -> p r c", r=HALF)
        for q in range(4):
            # GpSimd is ~2.6x slower per element than DVE here; give it one
            # slice (it is otherwise idle), DVE the rest
            eng = nc.gpsimd if q == 0 else nc.vector
            ra, rb = 7 * q, 7 * q + 7
            eng.tensor_tensor(
                prv[:, ra:rb, :],
                xpv[:, 1 + 2 * ra:1 + 2 * rb:2, :],
                xpv[:, 2 + 2 * ra:2 * rb + 1:2, :], ALU.add)
        for q in range(2):
            ra, rb = 14 * q, 14 * q + 14
            nc.vector.tensor_tensor(
                s4v[:, ra:rb, :], prv[:, ra:rb, 0:W:2],
                prv[:, ra:rb, 1:W:2], ALU.add)
        s4s[k] = s4

    for k in range(3):
        emit_load(k)
    emit_pre(0)

    pending = []  # deferred stage2/prelu2/store emitters from previous unit

    for k, (p, h) in enumerate(units):
        nA, nB = 2 * p, 2 * p + 1
        oy0 = HALF * h
        s4 = s4s.pop(k)
        spv = sp[h][:].rearrange("p (r c) -> p r c", r=SROWS)

        # ---- conv1: 9 taps x 4 chunks, A/B on concurrent 64x64 quads;
        # stage2 chunks of unit k-1 interleave to keep PE dense ----
        u = work3.tile([128, UN], f32, tag="u", name="u")
        out1 = work.tile([128, UN], bf16, tag="out1", name="out1")
        sg2 = work.tile([128, UN], bf16, tag="sg2", name="sg2")
        for half in range(2):
            # chunk-pair psum: chunk 2h at bank 0, chunk 2h+1 at bank 1
            cp = psum.tile([128, 1024], f32, tag="ps", name="ps")
            for cc in range(2):
                c = 2 * half + cc
                for t in range(9):
                    ky, kx = divmod(t, 3)
                    rs = ky + 14 * c
                    for i in range(2):
                        pr = slice(64 * i, 64 * i + 64)
                        rhs = spv[pr, rs:rs + 13:2, kx:kx + 111:2]
                        nc.tensor.matmul(
                            cp[pr, 512 * cc:512 * cc + CN],
                            wap(t)[pr, :], rhs,
                            start=(t == 0), stop=(t == 8),
                        )
                if pending:
                    pending.pop(0)()
            hs = slice(2 * CN * half, 2 * CN * (half + 1))
            cpv = cp[:].rearrange("p (i n) -> p i n", i=2)[:, :, 0:CN]
            uv = u[:, hs].rearrange("p (i n) -> p i n", i=2)
            s4h = s4[:, hs].rearrange("p (i n) -> p i n", i=2)
            nc.vector.scalar_tensor_tensor(
                uv, cpv, s3x4, s4h, ALU.mult, ALU.add)
            # prelu1/sign2 per half-unit: shorter chains into stage2
            nc.scalar.activation(
                out1[:, hs], u[:, hs], AF.Prelu,
                bias=pt[:, PB12:PB12 + 1], scale=0.25,
                alpha=pt[:, PA1:PA1 + 1])
            if trick_sign2:
                uhi = u[:, hs].bitcast(u16).rearrange(
                    "p (n two) -> p n two", two=2)[:, :, 1:2]
                nc.vector.tensor_scalar(
                    sg2[:, hs].bitcast(u16), uhi, 0x8000, 0x3F80,
                    ALU.bitwise_and, ALU.bitwise_or)
            elif fast_sign2:
                nc.scalar.activation(
                    sg2[:, hs], u[:, hs], AF.Sign,
                    bias=pt[:, PB12:PB12 + 1], scale=0.25)
            if half == 0:
                # after unit k's first-half epilogue: next unit's pre-work
                # (ACT sign1 queues behind prelu1(k,h0), not ahead of it)
                emit_pre(k + 1)
        while pending:
            pending.pop(0)()

        if has_b13:
            nc.vector.tensor_scalar(
                out1[:], out1[:], pt[:, PB13:PB13 + 1], None, ALU.add)
        if not fast_sign2 and not trick_sign2:
            nc.scalar.activation(
                sg2[:], out1[:], AF.Sign, bias=pt[:, PBS2:PBS2 + 1])

        emit_load(k + 3)

        # ---- stage 2 (deferred): per chunk a [128,1024] psum pair-tile
        # (A @ cols 0:CN, B @ cols 512:512+CN), matmuls interleaved A/B ----
        stg = work.tile([128, 2 * UN], f32, tag="stg", name="stg")

        def mk_stage2(c, k=k, out1=out1, sg2=sg2, stg=stg,
                      nA=nA, nB=nB, oy0=oy0):
            def emit():
                cs = slice(CN * c, CN * (c + 1))
                p2 = psum2.tile([128, 1024], f32, tag="ps2", name="ps2")
                for i in range(2):
                    pr = slice(64 * i, 64 * i + 64)
                    o2 = slice(512 * i, 512 * i + CN)
                    nc.tensor.matmul(
                        p2[:, o2], wt[pr, O_PW:O_PW + 128], sg2[pr, cs],
                        start=True, stop=False)
                for i in range(2):
                    pr = slice(64 * i, 64 * i + 64)
                    o2 = slice(512 * i, 512 * i + CN)
                    nc.tensor.matmul(
                        p2[:, o2], wt[pr, O_DIAG:O_DIAG + 128], out1[pr, cs],
                        start=False, stop=True)
                # one strided ACT Prelu over both images
                pin = p2[:].rearrange("p (i n) -> p i n", i=2)[:, :, 0:CN]
                pout = stg[:].rearrange("p (i n) -> p i n", i=2)[:, :, cs]
                nc.scalar.activation(
                    pout, pin, AF.Prelu,
                    bias=pt[:, PB22F:PB22F + 1],
                    scale=pt[:, PS2V:PS2V + 1],
                    alpha=pt[:, PA2F:PA2F + 1])
                if has_b23 and c == NCHUNK - 1:
                    nc.vector.tensor_scalar(
                        stg[:], stg[:], pt[:, PB23F:PB23F + 1],
                        None, ALU.add)
                # store per half-unit per image (overlap, good desc size);
                # with b23 the add covers all rows, so store only at the end
                if has_b23:
                    rr = (0, HALF) if c == NCHUNK - 1 else None
                else:
                    rr = {1: (0, 14), NCHUNK - 1: (14, HALF)}.get(c)
                if rr is not None:
                    for i, n in enumerate((nA, nB)):
                        sv = stg[:, UN * i:UN * (i + 1)].rearrange(
                            "p (r c) -> p r c", r=HALF)
                        nc.sync.dma_start(
                            y_d[n, :, oy0 + rr[0]:oy0 + rr[1], :],
                            sv[:, rr[0]:rr[1], :])
            return emit

        pending = [mk_stage2(c) for c in range(NCHUNK)]

    while pending:
        pending.pop(0)()

    for cm in reversed(pools):
        cm.__exit__(None, None, None)
    dram_cm.__exit__(None, None, None)
    tc_cm.__exit__(None, None, None)
    nc.compile()
    return nc, x_d.name, w_d.name, p_d.name, y_d.name


def _prep(inputs):
    f32 = np.float32
    bf = ml_dtypes.bfloat16
    w3 = np.asarray(inputs["w3"], f32)
    wpw1 = np.asarray(inputs["wpw1"], f32)
    wpw2 = np.asarray(inputs["wpw2"], f32)
    a1 = np.asarray(inputs["a1"], f32).reshape(CIN)
    a2 = np.asarray(inputs["a2"], f32).reshape(COUT)
    b11 = np.asarray(inputs["b11"], f32).reshape(CIN)
    b12 = np.asarray(inputs["b12"], f32).reshape(CIN)
    b13 = np.asarray(inputs["b13"], f32).reshape(CIN)
    b21 = np.asarray(inputs["b21"], f32).reshape(CIN)
    b22 = np.asarray(inputs["b22"], f32).reshape(COUT)
    b23 = np.asarray(inputs["b23"], f32).reshape(COUT)

    s3 = float(np.mean(np.abs(w3))) or 1.0
    s1 = float(np.mean(np.abs(wpw1))) or 1.0
    s2 = float(np.mean(np.abs(wpw2))) or 1.0

    # diag entries bf16(1/s_j); prelu2 scale 1/d_j compensates the rounding
    d1 = float(bf(1.0 / s1))
    d2 = float(bf(1.0 / s2))

    whalf = np.zeros((64, WCOLS), f32)
    sgn = np.sign
    for t in range(9):
        ky, kx = divmod(t, 3)
        whalf[:, 64 * t:64 * t + 64] = sgn(w3[:, :, ky, kx]).T
    whalf[:, O_PW:O_PW + 64] = sgn(wpw1[:, :, 0, 0]).T
    whalf[:, O_PW + 64:O_PW + 128] = sgn(wpw2[:, :, 0, 0]).T
    whalf[:, O_DIAG:O_DIAG + 64] = d1 * np.eye(64, dtype=f32)
    whalf[:, O_DIAG + 64:O_DIAG + 128] = d2 * np.eye(64, dtype=f32)
    wfull = np.concatenate([whalf, whalf], axis=0).astype(bf)

    def pairc(v):  # channel vec (64,) -> pair-layout (128,)
        return np.concatenate([v, v])

    params = np.zeros((128, NPARAM), f32)
    params[:, PA1] = pairc(a1)
    params[:, PB12] = pairc(b12)
    params[:, PB11] = pairc(b11)
    params[:, PA2F] = a2
    params[:, PB22F] = b22
    params[:, PS2V] = np.concatenate(
        [np.full(64, 1.0 / d1, f32), np.full(64, 1.0 / d2, f32)])
    params[:, PBS2] = pairc(b13 + b21)
    params[:, PB13] = pairc(b13)
    params[:, PB23F] = b23

    fast_sign2 = bool(np.all(b13 + b21 == 0.0) and np.all(a1 > 0))
    scal = {
        "s3x4": 4.0 * s3,
        "fast_sign2": fast_sign2,
        "trick_sign2": bool(fast_sign2 and np.all(b12 == 0.0)),
        "b11_zero": bool(np.all(b11 == 0.0)),
        "has_b13": bool(np.any(b13 != 0.0)),
        "has_b23": bool(np.any(b23 != 0.0)),
    }
    return wfull, params, scal


def kernel(**inputs):
    x = np.ascontiguousarray(np.asarray(inputs["x"], np.float32))
    wfull, params, scal = _prep(inputs)

    key = tuple(sorted(scal.items())) + (float(params.sum()),)
    if key not in _cache:
        _cache.clear()
        _cache[key] = _build(scal)
    nc, xn, wn, pn, yn = _cache[key]

    in_maps = []
    for i in range(NCORES):
        in_maps.append({
            xn: np.ascontiguousarray(x[BPC * i:BPC * (i + 1)]),
            wn: wfull,
            pn: params,
        })
    res = bass_utils.run_bass_kernel_spmd(nc, in_maps, core_ids=list(range(NCORES)))
    out = np.concatenate([res.results[i][yn] for i in range(NCORES)], axis=0)
    return out.astype(np.float32)
